# revision 1
# baseline (speedup 1.0000x reference)
"""Trainium2 Bass kernel for nn_CnnModel_70007966925195.

CNN backbone (3x conv1d+relu+maxpool2 -> mean -> FC+relu -> BN) followed by an
all-pairs contrastive loss. Data-parallel over N across 8 NeuronCores; z is
AllGathered and each core computes a 512x4096 row block of the loss matrix.

Layout strategy:
- conv1 (C_in=1, k=100): x stored transposed+padded as overlapping 128-row
  position chunks in SBUF; the tap-window select is folded into pre-shifted
  weight matrices (zero-padded to K=128). Two output positions (l, l+2) are
  packed into one M=128 matmul (cols 0-63 / 64-127), free dim = all 512
  local samples.
- conv2/conv3: position-streamed matmuls, one tap per matmul (K=ic),
  accumulating in PSUM; relu+maxpool eviction = ACT relu(bank0) -> SBUF,
  then DVE max(tmp, bank1) -> ring (max(relu(a),b) == relu(max(a,b))).
- Intermediates live in small ring buffers (parity-split partitions for h1).
- mean+FC fused: pooled conv3 tiles feed FC matmuls that accumulate over all
  64 positions in one PSUM bank (weights pre-scaled by 1/64).
- loss: d2 and y computed by accumulating matmuls (z.z via fp32r K=128 plus
  K=2 rank-2 terms), then clamp/sqrt/relu(1-d)/select, DMA out.
"""

import os
import sys

try:
    import concourse.bass as bass  # noqa: F401
except ImportError:
    sys.path.insert(0, "/opt/trn_rl_repo")

import numpy as np

import concourse.bass as bass  # noqa: F811
import concourse.mybir as mybir
import concourse.tile as tile
from concourse import bacc
from concourse.bass_utils import run_bass_kernel_spmd

F32 = mybir.dt.float32
F32R = mybir.dt.float32r
BF16 = mybir.dt.bfloat16
AL = mybir.AluOpType
ACT = mybir.ActivationFunctionType

N_CORES = 8
N = 4096
NL = N // N_CORES   # 512 samples per core
L = 512
K1, C1 = 100, 64          # conv1 kernel/outch
K2, C2 = 5, 128           # conv2
K3, C3 = 3, 256           # conv3
NCHUNK1 = 18              # conv1 x chunks, stride 29
SIG = 29                  # shift count (chunk stride)
T1 = 256                  # pooled conv1 positions
T2 = 128                  # pooled conv2 positions
T3 = 64                   # pooled conv3 positions
W1R = 8                   # h1 ring depth (pairs)
W2R = 8                   # h2 ring depth

LAST_RESULT = None        # BassKernelResults stash for test harness


def build_nc():
    kdebug = os.environ.get("KDEBUG", "full")
    nc = bacc.Bacc("TRN2", target_bir_lowering=False, debug=False,
                   num_devices=N_CORES)

    xs_d = nc.dram_tensor("xs", [NCHUNK1, 128, NL], BF16, kind="ExternalInput")
    w1s_d = nc.dram_tensor("w1s", [31, 128, 128], BF16, kind="ExternalInput")
    w2d_d = nc.dram_tensor("w2d", [6, 128, C2], BF16, kind="ExternalInput")
    w3t_d = nc.dram_tensor("w3t", [K3, 128, C3], BF16, kind="ExternalInput")
    fcw_d = nc.dram_tensor("fcw", [2, 128, 128], BF16, kind="ExternalInput")
    fcb_d = nc.dram_tensor("fcb", [128, 1], F32, kind="ExternalInput")
    bna_d = nc.dram_tensor("bna", [128, 1], F32, kind="ExternalInput")
    bnb_d = nc.dram_tensor("bnb", [128, 1], F32, kind="ExternalInput")
    abl_d = nc.dram_tensor("abl", [2, NL], BF16, kind="ExternalInput")
    abf_d = nc.dram_tensor("abf", [2, N], BF16, kind="ExternalInput")
    onc_d = nc.dram_tensor("onc", [128, 1], F32, kind="ExternalInput")
    onr_d = nc.dram_tensor("onr", [1, N], F32, kind="ExternalInput")
    out_d = nc.dram_tensor("out", [NL, N], F32, kind="ExternalOutput")
    gin_d = nc.dram_tensor("gin", [129, NL], F32, kind="Internal")
    gout_d = nc.dram_tensor("gout", [N_CORES, 129, NL], F32, kind="Internal",
                            addr_space="Shared")

    with tile.TileContext(nc) as tc:
        with (
            tc.tile_pool(name="const", bufs=1) as cpool,
            tc.tile_pool(name="zbuf", bufs=1) as zpool,
            tc.tile_pool(name="fcp", bufs=1, space="PSUM") as fcpool,
        ):
            # ---- persistent SBUF tensors ----
            xs = cpool.tile([128, NCHUNK1, NL], BF16, tag="xs")
            nc.sync.dma_start(xs[:], xs_d[:].rearrange("c p n -> p c n"))
            w1s = cpool.tile([128, 31, 128], BF16, tag="w1s")
            nc.sync.dma_start(w1s[:], w1s_d[:].rearrange("s k o -> k s o"))
            w2d = cpool.tile([128, 6, C2], BF16, tag="w2d")
            nc.sync.dma_start(w2d[:], w2d_d[:].rearrange("t k o -> k t o"))
            w3t = cpool.tile([128, K3, C3], BF16, tag="w3t")
            nc.sync.dma_start(w3t[:], w3t_d[:].rearrange("t k o -> k t o"))
            fcw = cpool.tile([128, 2, 128], BF16, tag="fcw")
            nc.sync.dma_start(fcw[:], fcw_d[:].rearrange("c k o -> k c o"))
            fcb = cpool.tile([128, 1], F32, tag="fcb")
            nc.sync.dma_start(fcb[:], fcb_d[:])
            bna = cpool.tile([128, 1], F32, tag="bna")
            nc.sync.dma_start(bna[:], bna_d[:])
            bnb = cpool.tile([128, 1], F32, tag="bnb")
            nc.sync.dma_start(bnb[:], bnb_d[:])
            abl = cpool.tile([2, NL], BF16, tag="abl")
            nc.sync.dma_start(abl[:], abl_d[:])
            abf = cpool.tile([2, N], BF16, tag="abf")
            nc.sync.dma_start(abf[:], abf_d[:])

            h1r = cpool.tile([128, W1R, NL], BF16, tag="h1r")  # (parity,ic), u
            h2r = cpool.tile([128, W2R, NL], BF16, tag="h2r")
            fc_ps = fcpool.tile([128, NL], F32, tag="fc")

            # ---- fused conv pipeline ----
            with (
                tc.tile_pool(name="p1", bufs=1, space="PSUM") as p1pool,
                tc.tile_pool(name="p2", bufs=1, space="PSUM") as p2pool,
                tc.tile_pool(name="p3", bufs=1, space="PSUM") as p3pool,
                tc.tile_pool(name="h3", bufs=2) as h3pool,
                tc.tile_pool(name="ev", bufs=3) as evpool,
            ):
                def conv1_batch(i1):
                    # positions l = 4*i1 .. 4*i1+3 -> pooled t = 2*i1, 2*i1+1
                    # bank j: partitions 0-63 = pos 4i+j, 64-127 = pos 4i+2+j
                    ps = p1pool.tile([128, 2, NL], F32, tag="p1")
                    for j in range(2):
                        la = 4 * i1 + j
                        lb = la + 2
                        ca, sa = divmod(la, SIG)
                        cb, sb_ = divmod(lb, SIG)
                        if ca == cb:
                            nc.tensor.matmul(
                                ps[:, j, :], w1s[:, sa, :], xs[:, ca, :],
                                start=True, stop=True)
                        else:
                            nc.tensor.matmul(
                                ps[:, j, :], w1s[:, sa, :], xs[:, ca, :],
                                start=True, stop=False)
                            nc.tensor.matmul(
                                ps[:, j, :], w1s[:, 29 + sb_, :],
                                xs[:, cb, :],
                                start=False, stop=True)
                    tmp = evpool.tile([128, NL], F32, tag="ev")
                    nc.scalar.activation(tmp[:], ps[:, 0, :], ACT.Relu)
                    nc.vector.tensor_max(h1r[:, i1 % W1R, :], tmp[:],
                                         ps[:, 1, :])

                def conv2_pair(j2):
                    # output positions l2 = 2*j2, 2*j2+1 -> pooled t2 = j2.
                    # Each matmul contracts one full h1 ring slot (K=128 =
                    # even-parity tap on rows 0-63, odd on 64-127); the tap
                    # windowing is baked into 6 weight variants.  All
                    # operands at base partition 0 (mixed row-group fp32r
                    # accumulation faults on HW).
                    ps = p2pool.tile([128, 2, NL], F32, tag="p2")
                    for jj in range(2):
                        l2 = 2 * j2 + jj
                        if l2 % 2 == 0:
                            mlist = [((l2 - 2) // 2, 0), (l2 // 2, 1),
                                     ((l2 + 2) // 2, 2)]
                        else:
                            mlist = [((l2 - 3) // 2, 3), ((l2 - 1) // 2, 4),
                                     ((l2 + 1) // 2, 5)]
                        mlist = [(u, v) for u, v in mlist if 0 <= u < T1 // 2]
                        for ti, (u, v) in enumerate(mlist):
                            nc.tensor.matmul(
                                ps[:, jj, :],
                                w2d[:, v, :],
                                h1r[:, u % W1R, :],
                                start=(ti == 0), stop=(ti == len(mlist) - 1),
                            )
                    tmp = evpool.tile([128, NL], F32, tag="ev")
                    nc.scalar.activation(tmp[:], ps[:, 0, :], ACT.Relu)
                    nc.vector.tensor_max(h2r[:, j2 % W2R, :], tmp[:],
                                         ps[:, 1, :])

                def conv3_pair(j3):
                    # output positions l3 = 2*j3, 2*j3+1 -> pooled t3 = j3
                    for ch in range(2):
                        ps = p3pool.tile([128, 2, NL], F32, tag="p3")
                        for jj in range(2):
                            l3 = 2 * j3 + jj
                            taps = [t for t in range(K3)
                                    if 0 <= l3 + t - 1 < 2 * T2]
                            for ti, t in enumerate(taps):
                                t2 = l3 + t - 1
                                nc.tensor.matmul(
                                    ps[:, jj, :],
                                    w3t[:, t, 128 * ch:128 * ch + 128],
                                    h2r[:, t2 % W2R, :],
                                    start=(ti == 0),
                                    stop=(ti == len(taps) - 1),
                                )
                        h3t = h3pool.tile([128, NL], BF16, tag="h3t")
                        tmp = evpool.tile([128, NL], F32, tag="ev")
                        nc.scalar.activation(tmp[:], ps[:, 0, :], ACT.Relu)
                        nc.vector.tensor_max(h3t[:], tmp[:], ps[:, 1, :])
                        nc.tensor.matmul(
                            fc_ps[:],
                            fcw[:, ch, :],
                            h3t[:],
                            start=(j3 == 0 and ch == 0),
                            stop=(j3 == T3 - 1 and ch == 1),
                            skip_group_check=True,
                        )

                kph = os.environ.get("KPHASES", "123")
                for ii in range(132):
                    if ii < 128 and "1" in kph:
                        conv1_batch(ii)
                    if 2 <= ii < 130 and "2" in kph:
                        conv2_pair(ii - 2)
                    if (ii >= 4 and ii % 2 == 0 and (ii - 4) // 2 < T3
                            and "3" in kph):
                        conv3_pair((ii - 4) // 2)
                if "3" not in kph:
                    # fc_ps never written; give it a defined value
                    nc.tensor.matmul(fc_ps[:], fcw[:, 0, :],
                                     h2r[:, 0, :] if "2" in kph
                                     else h1r[:, 0, :],
                                     start=True, stop=True)

            if kdebug == "convs":
                dbg = zpool.tile([128, NL], F32, tag="zT")
                nc.vector.tensor_copy(dbg[:], fc_ps[:])
                nc.sync.dma_start(out_d[0:128, 0:NL], dbg[:])
            else:
                _emit_tail(nc, tc, zpool, fc_ps, fcb, bna, bnb, abl, abf,
                           onc_d, onr_d, out_d, gin_d, gout_d, kdebug)

    nc.compile()
    return nc


def _emit_tail(nc, tc, zpool, fc_ps, fcb, bna, bnb, abl, abf,
               onc_d, onr_d, out_d, gin_d, gout_d, kdebug):
    # ---- z = BN(relu(FC)) ; gather z + |z|^2 ----
    with tc.tile_pool(name="sqp", bufs=1, space="PSUM") as sqpool:
        zT = zpool.tile([128, NL], F32, tag="zT")
        nc.scalar.activation(zT[:], fc_ps[:], ACT.Relu,
                             bias=fcb[:], scale=1.0)
        nc.vector.tensor_scalar(zT[:], zT[:], bna[:], bnb[:],
                                op0=AL.mult, op1=AL.add)
        zsq = zpool.tile([128, NL], F32, tag="zsq")
        nc.scalar.activation(zsq[:], zT[:], ACT.Square)
        ones_col = zpool.tile([128, 1], F32, tag="ones_col")
        nc.sync.dma_start(ones_col[:], onc_d[:])
        sq_ps = sqpool.tile([1, NL], F32, tag="sq")
        nc.tensor.matmul(sq_ps[:], ones_col[:], zsq[:],
                         start=True, stop=True)
        sqones = zpool.tile([2, NL], F32, tag="sqones")
        nc.sync.dma_start(sqones[1:2, :], onr_d[0:1, 0:NL])
        nc.vector.tensor_copy(sqones[0:1, :], sq_ps[:])
        zm2 = zpool.tile([128, NL], F32, tag="zm2")
        nc.vector.tensor_scalar_mul(zm2[:], zT[:], -2.0)

        if kdebug == "z":
            nc.sync.dma_start(out_d[0:128, 0:NL], zT[:])
            return

        nc.sync.dma_start(gin_d[0:128, :], zT[:])
        nc.sync.dma_start(gin_d[128:129, :], sqones[0:1, :])
        nc.gpsimd.collective_compute(
            "AllGather", AL.bypass,
            replica_groups=[list(range(N_CORES))],
            ins=[gin_d[:]], outs=[gout_d[:]],
        )

        zfT = zpool.tile([128, N_CORES, NL], F32, tag="zfT")
        nc.sync.dma_start(
            zfT[:], gout_d[:, 0:128, :].rearrange("r p n -> p r n"))
        onesqf = zpool.tile([2, N], F32, tag="onesqf")
        nc.sync.dma_start(onesqf[0:1, :], onr_d[:])
        nc.sync.dma_start(
            onesqf[1:2, :].rearrange("p (r n) -> p r n", r=N_CORES),
            gout_d[:, 128:129, :].rearrange("r p n -> p r n"))

        zm2h = zpool.tile([128, NL], BF16, tag="zm2h")
        nc.vector.tensor_copy(zm2h[:], zm2[:])
        zm2l = zpool.tile([128, NL], BF16, tag="zm2l")
        nc.vector.tensor_sub(zm2l[:], zm2[:], zm2h[:])
        zfh = zpool.tile([128, N_CORES, NL], BF16, tag="zfh")
        nc.vector.tensor_copy(zfh[:], zfT[:])
        zfl = zpool.tile([128, N_CORES, NL], BF16, tag="zfl")
        nc.vector.tensor_sub(zfl[:], zfT[:], zfh[:])

        if kdebug == "gather":
            zfc = zpool.tile([128, NL], F32, tag="zfc")
            nc.vector.tensor_copy(zfc[:], zfT[:, 0, :])
            nc.sync.dma_start(out_d[0:128, 0:NL], zfc[:])
            return

        # ---- loss row block ----
        with (
            tc.tile_pool(name="pd", bufs=2, space="PSUM") as pdpool,
            tc.tile_pool(name="py", bufs=2, space="PSUM") as pypool,
            tc.tile_pool(name="lw", bufs=4) as lwpool,
        ):
            for rb in range(4):
                rs = slice(128 * rb, 128 * rb + 128)
                for jc in range(N_CORES):
                    js = slice(NL * jc, NL * jc + NL)
                    pd = pdpool.tile([128, NL], F32, tag="pd")
                    py = pypool.tile([128, NL], F32, tag="py")
                    nc.tensor.matmul(pd[:], zm2h[:, rs], zfh[:, jc, :],
                                     start=True, stop=False)
                    nc.tensor.matmul(pd[:], zm2h[:, rs], zfl[:, jc, :],
                                     start=False, stop=False)
                    nc.tensor.matmul(pd[:], zm2l[:, rs], zfh[:, jc, :],
                                     start=False, stop=False)
                    nc.tensor.matmul(pd[:], sqones[:, rs], onesqf[:, js],
                                     start=False, stop=True)
                    nc.tensor.matmul(py[:], abl[:, rs], abf[:, js],
                                     start=True, stop=True)
                    c2 = lwpool.tile([128, NL], F32, tag="c2")
                    nc.vector.tensor_scalar_max(c2[:], pd[:], 0.0)
                    dd = lwpool.tile([128, NL], F32, tag="dd")
                    nc.scalar.activation(dd[:], c2[:], ACT.Sqrt)
                    tt = lwpool.tile([128, NL], F32, tag="tt")
                    nc.scalar.activation(tt[:], dd[:], ACT.Relu,
                                         bias=1.0, scale=-1.0)
                    cl = lwpool.tile([128, NL], F32, tag="cl")
                    nc.vector.select(
                        cl[:], py[:].bitcast(mybir.dt.int32),
                        dd[:], tt[:])
                    nc.sync.dma_start(out_d[rs, js], cl[:])


def _prep_inputs(samples, samples_info, conv1_w, conv1_b, conv2_w, conv2_b,
                 conv3_w, conv3_b, fc_w, fc_b, bn_gamma, bn_beta, bn_mean,
                 bn_var):
    f = np.float32
    samples = np.asarray(samples, f)
    info = np.asarray(samples_info, f)
    conv1_w = np.asarray(conv1_w, f)
    conv2_w = np.asarray(conv2_w, f)
    conv3_w = np.asarray(conv3_w, f)

    assert np.all(np.asarray(conv1_b) == 0), "conv1_b != 0 unsupported"
    assert np.all(np.asarray(conv2_b) == 0), "conv2_b != 0 unsupported"
    assert np.all(np.asarray(conv3_b) == 0), "conv3_b != 0 unsupported"

    # conv1 shifted weights, position pairs (l, l+2) packed into M=128:
    # cols 0-63 use shift s, cols 64-127 use shift s+2.  Indices 27/28 are
    # the left-only (shift 27/28) variants, 29/30 right-only (shift 0/1)
    # for pairs whose two windows land in adjacent x chunks.
    w1b = np.zeros((SIG, 128, C1), f)
    for s in range(SIG):
        w1b[s, s:s + K1, :] = conv1_w[:, 0, :].T
    w1s = np.zeros((31, 128, 128), f)
    for s in range(27):
        w1s[s, :, 0:64] = w1b[s]
        w1s[s, :, 64:128] = w1b[s + 2]
    for d in range(2):
        w1s[27 + d, :, 0:64] = w1b[27 + d]
        w1s[29 + d, :, 64:128] = w1b[d]
    # conv2 tap-pair weight variants (top rows 0-63 = even-parity tap,
    # bottom rows 64-127 = odd-parity tap of the same h1 pair slot):
    # even l2: V0=[t0;t1] V1=[t2;t3] V2=[t4;0]
    # odd  l2: V3=[0;t0]  V4=[t1;t2] V5=[t3;t4]
    w2t = [conv2_w[:, :, t].T for t in range(K2)]   # [64 ic, 128 oc]
    w2d = np.zeros((6, 128, C2), f)
    pairs = [(0, 1), (2, 3), (4, None), (None, 0), (1, 2), (3, 4)]
    for v, (top, bot) in enumerate(pairs):
        if top is not None:
            w2d[v, 0:64, :] = w2t[top]
        if bot is not None:
            w2d[v, 64:128, :] = w2t[bot]
    w3tt = np.zeros((K3, 128, C3), f)
    for t in range(K3):
        w3tt[t] = conv3_w[:, :, t].T   # [128 ic, 256 oc]
    fcw = np.zeros((2, 128, 128), f)
    fcwT = np.asarray(fc_w, f).T / f(T3)   # [256, 128]
    fcw[0] = fcwT[0:128, :]
    fcw[1] = fcwT[128:256, :]
    fcb = np.asarray(fc_b, f).reshape(128, 1)
    bna = (np.asarray(bn_gamma, f) /
           np.sqrt(np.asarray(bn_var, f) + f(1e-5))).reshape(128, 1)
    bnb = (np.asarray(bn_beta, f) -
           np.asarray(bn_mean, f).reshape(128) * bna[:, 0]).reshape(128, 1)

    writer, gen = info[:, 0], info[:, 1]
    assert np.all((writer == 0) | (writer == 1)), "non-binary writer id"
    a_full = (gen * (1.0 - writer)).astype(f)
    b_full = (gen * writer).astype(f)
    abf = np.stack([a_full, b_full])          # [2, N]

    import ml_dtypes
    bf = ml_dtypes.bfloat16
    w1s_b = w1s.astype(bf)
    w2d_b = w2d.astype(bf)
    w3t_b = w3tt.astype(bf)
    fcw_b = fcw.astype(bf)

    ones_col_np = np.ones((128, 1), f)
    ones_row_np = np.ones((1, N), f)

    # x transposed, padded (49 left / 50 right + tail), cut into 18
    # overlapping 128-row chunks at stride 29
    in_maps = []
    for core in range(N_CORES):
        n0 = core * NL
        xpad = np.zeros((624, NL), f)
        xpad[49:49 + L, :] = samples[n0:n0 + NL, 0, :].T
        xsc = np.zeros((NCHUNK1, 128, NL), f)
        for c in range(NCHUNK1):
            xsc[c] = xpad[SIG * c:SIG * c + 128, :]
        in_maps.append({
            "xs": xsc.astype(bf), "onc": ones_col_np, "onr": ones_row_np,
            "w1s": w1s_b, "w2d": w2d_b, "w3t": w3t_b, "fcw": fcw_b,
            "fcb": fcb,
            "bna": bna, "bnb": bnb,
            "abl": np.ascontiguousarray(abf[:, n0:n0 + NL]).astype(bf),
            "abf": abf.astype(bf),
        })
    return in_maps


def kernel(**inputs):
    global LAST_RESULT
    in_maps = _prep_inputs(**inputs)
    nc = build_nc()
    res = run_bass_kernel_spmd(nc, in_maps, core_ids=list(range(N_CORES)))
    LAST_RESULT = res
    out = np.concatenate([r["out"] for r in res.results], axis=0)
    np.fill_diagonal(out, 0.0)
    return out.astype(np.float32)



# revision 2
# speedup vs baseline: 1.4040x; 1.4040x over previous
"""Trainium2 Bass kernel for nn_CnnModel_70007966925195.

CNN backbone (3x conv1d+relu+maxpool2 -> mean -> FC+relu -> BN) followed by an
all-pairs contrastive loss. Data-parallel over N across 8 NeuronCores; z is
AllGathered and each core computes a 512x4096 row block of the loss matrix.

Layout strategy:
- conv1 (C_in=1, k=100): x stored transposed+padded as overlapping 128-row
  position chunks in SBUF; the tap-window select is folded into pre-shifted
  weight matrices (zero-padded to K=128). Two output positions (l, l+2) are
  packed into one M=128 matmul (cols 0-63 / 64-127), free dim = all 512
  local samples.
- conv2/conv3: position-streamed matmuls, one tap per matmul (K=ic),
  accumulating in PSUM; relu+maxpool eviction = ACT relu(bank0) -> SBUF,
  then DVE max(tmp, bank1) -> ring (max(relu(a),b) == relu(max(a,b))).
  conv3 is emitted as half-pairs (one ch-half per pipeline step) with a
  double-buffered PSUM pool so evictions never stall the PE.
- mean+FC: h3 tiles are summed into an SBUF accumulator on the otherwise
  idle Pool engine (weights pre-scaled by 1/64); FC itself is 2 matmuls at
  the end.  This frees a PSUM bank and kills the FC->eviction stall chain.
- loss: d2 computed per 128x512 tile as ONE fp32r K=128 matmul (z.z) plus
  one fp32r K=2 rank-2 matmul (sq_i + sq_j); y-mask via bf16 K=2 matmul;
  then clamp/sqrt/relu(1-d)/select, DMA out.  fp32r needs operands rounded
  by a DVE/ACT producer (or DMA of pre-rounded data) - verified on HW.
"""

import os
import sys

try:
    import concourse.bass as bass  # noqa: F401
except ImportError:
    sys.path.insert(0, "/opt/trn_rl_repo")

import numpy as np

import concourse.bass as bass  # noqa: F811
import concourse.mybir as mybir
import concourse.tile as tile
from concourse import bacc
from concourse.bass_utils import run_bass_kernel_spmd

F32 = mybir.dt.float32
F32R = mybir.dt.float32r
BF16 = mybir.dt.bfloat16
AL = mybir.AluOpType
ACT = mybir.ActivationFunctionType

N_CORES = 8
N = 4096
NL = N // N_CORES   # 512 samples per core
L = 512
K1, C1 = 100, 64          # conv1 kernel/outch
K2, C2 = 5, 128           # conv2
K3, C3 = 3, 256           # conv3
NCHUNK1 = 18              # conv1 x chunks, stride 29
SIG = 29                  # shift count (chunk stride)
T1 = 256                  # pooled conv1 positions
T2 = 128                  # pooled conv2 positions
T3 = 64                   # pooled conv3 positions
W1R = 8                   # h1 ring depth (pairs)
W2R = 8                   # h2 ring depth

LAST_RESULT = None        # BassKernelResults stash for test harness


def build_nc():
    kdebug = os.environ.get("KDEBUG", "full")
    nc = bacc.Bacc("TRN2", target_bir_lowering=False, debug=False,
                   num_devices=N_CORES)

    xs_d = nc.dram_tensor("xs", [NCHUNK1, 128, NL], BF16, kind="ExternalInput")
    w1s_d = nc.dram_tensor("w1s", [31, 128, 128], BF16, kind="ExternalInput")
    w2d_d = nc.dram_tensor("w2d", [6, 128, C2], BF16, kind="ExternalInput")
    w3t_d = nc.dram_tensor("w3t", [K3, 128, C3], BF16, kind="ExternalInput")
    fcw_d = nc.dram_tensor("fcw", [2, 128, 128], BF16, kind="ExternalInput")
    fcb_d = nc.dram_tensor("fcb", [128, 1], F32, kind="ExternalInput")
    bna_d = nc.dram_tensor("bna", [128, 1], F32, kind="ExternalInput")
    bnb_d = nc.dram_tensor("bnb", [128, 1], F32, kind="ExternalInput")
    abl_d = nc.dram_tensor("abl", [2, NL], BF16, kind="ExternalInput")
    abf_d = nc.dram_tensor("abf", [2, N], BF16, kind="ExternalInput")
    onc_d = nc.dram_tensor("onc", [128, 1], F32R, kind="ExternalInput")
    onr_d = nc.dram_tensor("onr", [1, N], F32R, kind="ExternalInput")
    out_d = nc.dram_tensor("out", [NL, N], F32, kind="ExternalOutput")
    gin_d = nc.dram_tensor("gin", [129, NL], F32R, kind="Internal")
    gout_d = nc.dram_tensor("gout", [N_CORES, 129, NL], F32R, kind="Internal",
                            addr_space="Shared")

    with tile.TileContext(nc) as tc:
        with (
            tc.tile_pool(name="const", bufs=1) as cpool,
            tc.tile_pool(name="zbuf", bufs=1) as zpool,
        ):
            # ---- persistent SBUF tensors ----
            w1s = cpool.tile([128, 31, 128], BF16, tag="w1s")
            nc.sync.dma_start(w1s[:], w1s_d[:].rearrange("s k o -> k s o"))
            xs = cpool.tile([128, NCHUNK1, NL], BF16, tag="xs")
            for c in range(NCHUNK1):
                nc.sync.dma_start(
                    xs[:, c, :],
                    xs_d[c, :, :])
            w2d = cpool.tile([128, 6, C2], BF16, tag="w2d")
            nc.sync.dma_start(w2d[:], w2d_d[:].rearrange("t k o -> k t o"))
            w3t = cpool.tile([128, K3, C3], BF16, tag="w3t")
            nc.sync.dma_start(w3t[:], w3t_d[:].rearrange("t k o -> k t o"))
            fcw = cpool.tile([128, 2, 128], BF16, tag="fcw")
            nc.sync.dma_start(fcw[:], fcw_d[:].rearrange("c k o -> k c o"))
            fcb = cpool.tile([128, 1], F32, tag="fcb")
            nc.sync.dma_start(fcb[:], fcb_d[:])
            bna = cpool.tile([128, 1], F32, tag="bna")
            nc.sync.dma_start(bna[:], bna_d[:])
            bnb = cpool.tile([128, 1], F32, tag="bnb")
            nc.sync.dma_start(bnb[:], bnb_d[:])
            abl = cpool.tile([2, NL], BF16, tag="abl")
            nc.sync.dma_start(abl[:], abl_d[:])
            abf = cpool.tile([2, N], BF16, tag="abf")
            nc.sync.dma_start(abf[:], abf_d[:])

            h1r = cpool.tile([128, W1R, NL], BF16, tag="h1r")  # (parity,ic), u
            h2r = cpool.tile([128, W2R, NL], BF16, tag="h2r")
            hsum = cpool.tile([128, 2, NL], F32, tag="hsum")
            nc.gpsimd.memset(hsum[:], 0.0)

            # ---- fused conv pipeline ----
            with (
                tc.tile_pool(name="p1", bufs=1, space="PSUM") as p1pool,
                tc.tile_pool(name="p2", bufs=1, space="PSUM") as p2pool,
                tc.tile_pool(name="p3", bufs=2, space="PSUM") as p3pool,
                tc.tile_pool(name="h3", bufs=3) as h3pool,
                tc.tile_pool(name="ev", bufs=4) as evpool,
            ):
                def conv1_batch(i1):
                    # positions l = 4*i1 .. 4*i1+3 -> pooled t = 2*i1, 2*i1+1
                    # bank j: partitions 0-63 = pos 4i+j, 64-127 = pos 4i+2+j
                    ps = p1pool.tile([128, 2, NL], F32, tag="p1")
                    for j in range(2):
                        la = 4 * i1 + j
                        lb = la + 2
                        ca, sa = divmod(la, SIG)
                        cb, sb_ = divmod(lb, SIG)
                        if ca == cb:
                            nc.tensor.matmul(
                                ps[:, j, :], w1s[:, sa, :], xs[:, ca, :],
                                start=True, stop=True)
                        else:
                            nc.tensor.matmul(
                                ps[:, j, :], w1s[:, sa, :], xs[:, ca, :],
                                start=True, stop=False)
                            nc.tensor.matmul(
                                ps[:, j, :], w1s[:, 29 + sb_, :],
                                xs[:, cb, :],
                                start=False, stop=True)
                    tmp = evpool.tile([128, NL], F32, tag="ev")
                    nc.scalar.activation(tmp[:], ps[:, 0, :], ACT.Relu)
                    nc.vector.tensor_max(h1r[:, i1 % W1R, :], tmp[:],
                                         ps[:, 1, :])

                def conv2_pair(j2):
                    # output positions l2 = 2*j2, 2*j2+1 -> pooled t2 = j2.
                    # Each matmul contracts one full h1 ring slot (K=128 =
                    # even-parity tap on rows 0-63, odd on 64-127); the tap
                    # windowing is baked into 6 weight variants.
                    ps = p2pool.tile([128, 2, NL], F32, tag="p2")
                    for jj in range(2):
                        l2 = 2 * j2 + jj
                        if l2 % 2 == 0:
                            mlist = [((l2 - 2) // 2, 0), (l2 // 2, 1),
                                     ((l2 + 2) // 2, 2)]
                        else:
                            mlist = [((l2 - 3) // 2, 3), ((l2 - 1) // 2, 4),
                                     ((l2 + 1) // 2, 5)]
                        mlist = [(u, v) for u, v in mlist if 0 <= u < T1 // 2]
                        for ti, (u, v) in enumerate(mlist):
                            nc.tensor.matmul(
                                ps[:, jj, :],
                                w2d[:, v, :],
                                h1r[:, u % W1R, :],
                                start=(ti == 0), stop=(ti == len(mlist) - 1),
                            )
                    tmp = evpool.tile([128, NL], F32, tag="ev")
                    nc.scalar.activation(tmp[:], ps[:, 0, :], ACT.Relu)
                    nc.vector.tensor_max(h2r[:, j2 % W2R, :], tmp[:],
                                         ps[:, 1, :])

                def conv3_half(j3, ch):
                    # output positions l3 = 2*j3, 2*j3+1 -> pooled t3 = j3,
                    # out-channel half ch.  Pooled h3 is accumulated into
                    # hsum on the Pool engine (FC mean path).
                    ps = p3pool.tile([128, 2, NL], F32, tag="p3")
                    for jj in range(2):
                        l3 = 2 * j3 + jj
                        taps = [t for t in range(K3)
                                if 0 <= l3 + t - 1 < 2 * T2]
                        for ti, t in enumerate(taps):
                            t2 = l3 + t - 1
                            nc.tensor.matmul(
                                ps[:, jj, :],
                                w3t[:, t, 128 * ch:128 * ch + 128],
                                h2r[:, t2 % W2R, :],
                                start=(ti == 0),
                                stop=(ti == len(taps) - 1),
                            )
                    h3t = h3pool.tile([128, NL], BF16, tag="h3t")
                    tmp = evpool.tile([128, NL], F32, tag="ev")
                    nc.scalar.activation(tmp[:], ps[:, 0, :], ACT.Relu)
                    nc.vector.tensor_max(h3t[:], tmp[:], ps[:, 1, :])
                    nc.gpsimd.tensor_tensor(hsum[:, ch, :], hsum[:, ch, :],
                                            h3t[:], AL.add)

                for ii in range(132):
                    if ii < 128:
                        conv1_batch(ii)
                    if 2 <= ii < 130:
                        conv2_pair(ii - 2)
                    if ii >= 4:
                        j3, ch = divmod(ii - 4, 2)
                        if j3 < T3:
                            conv3_half(j3, ch)

            _emit_tail(nc, tc, zpool, hsum, fcw, fcb, bna, bnb, abl, abf,
                       onc_d, onr_d, out_d, gin_d, gout_d, kdebug)

    nc.compile()
    return nc


def _emit_tail(nc, tc, zpool, hsum, fcw, fcb, bna, bnb, abl, abf,
               onc_d, onr_d, out_d, gin_d, gout_d, kdebug):
    # ---- FC from pooled means, z = BN(relu(FC)) ; gather z + |z|^2 ----
    with (
        tc.tile_pool(name="fcp", bufs=1, space="PSUM") as fcpool,
        tc.tile_pool(name="sqp", bufs=1, space="PSUM") as sqpool,
    ):
        hsb = zpool.tile([128, 2, NL], BF16, tag="hsb")
        nc.vector.tensor_copy(hsb[:], hsum[:])
        fc_ps = fcpool.tile([128, NL], F32, tag="fc")
        nc.tensor.matmul(fc_ps[:], fcw[:, 0, :], hsb[:, 0, :],
                         start=True, stop=False)
        nc.tensor.matmul(fc_ps[:], fcw[:, 1, :], hsb[:, 1, :],
                         start=False, stop=True)

        zT = zpool.tile([128, NL], F32, tag="zT")
        nc.scalar.activation(zT[:], fc_ps[:], ACT.Relu,
                             bias=fcb[:], scale=1.0)
        nc.vector.tensor_scalar(zT[:], zT[:], bna[:], bnb[:],
                                op0=AL.mult, op1=AL.add)

        if kdebug == "z":
            nc.sync.dma_start(out_d[0:128, 0:NL], zT[:])
            return

        # f32r copies of z and -2z (DVE rounds to f32r encoding)
        zr = zpool.tile([128, NL], F32R, tag="zr")
        nc.vector.tensor_copy(zr[:], zT[:])
        nc.sync.dma_start(gin_d[0:128, :], zr[:])
        zm2r = zpool.tile([128, NL], F32R, tag="zm2r")
        nc.vector.tensor_scalar_mul(zm2r[:], zT[:], -2.0)

        # |z|^2 row: ones^T (z*z), then round to f32r
        zsq = zpool.tile([128, NL], F32, tag="zsq")
        nc.scalar.activation(zsq[:], zT[:], ACT.Square)
        zsqr = zpool.tile([128, NL], F32R, tag="zsqr")
        nc.vector.tensor_copy(zsqr[:], zsq[:])
        ones_col = zpool.tile([128, 1], F32R, tag="ones_col")
        nc.sync.dma_start(ones_col[:], onc_d[:])
        sq_ps = sqpool.tile([1, NL], F32, tag="sq")
        nc.tensor.matmul(sq_ps[:], ones_col[:], zsqr[:],
                         start=True, stop=True)
        # sqones rows: [sq_local ; ones] (f32r)
        sqones = zpool.tile([2, NL], F32R, tag="sqones")
        nc.vector.tensor_copy(sqones[0:1, :], sq_ps[:])
        nc.sync.dma_start(sqones[1:2, :], onr_d[0:1, 0:NL])
        nc.sync.dma_start(gin_d[128:129, :], sqones[0:1, :])

        nc.gpsimd.collective_compute(
            "AllGather", AL.bypass,
            replica_groups=[list(range(N_CORES))],
            ins=[gin_d[:]], outs=[gout_d[:]],
        )

        zfT = zpool.tile([128, N_CORES, NL], F32R, tag="zfT")
        nc.sync.dma_start(
            zfT[:], gout_d[:, 0:128, :].rearrange("r p n -> p r n"))
        # onesqf rows: [ones ; sq_full] (f32r)
        onesqf = zpool.tile([2, N], F32R, tag="onesqf")
        nc.sync.dma_start(onesqf[0:1, :], onr_d[:])
        nc.sync.dma_start(
            onesqf[1:2, :].rearrange("p (r n) -> p r n", r=N_CORES),
            gout_d[:, 128:129, :].rearrange("r p n -> p r n"))

        if kdebug == "gather":
            zfc = zpool.tile([128, NL], F32, tag="zfc")
            nc.vector.tensor_copy(zfc[:], zfT[:, 0, :])
            nc.sync.dma_start(out_d[0:128, 0:NL], zfc[:])
            return

        # ---- loss row block ----
        with (
            tc.tile_pool(name="pd", bufs=2, space="PSUM") as pdpool,
            tc.tile_pool(name="py", bufs=2, space="PSUM") as pypool,
            tc.tile_pool(name="lw", bufs=4) as lwpool,
        ):
            for rb in range(4):
                rs = slice(128 * rb, 128 * rb + 128)
                for jc in range(N_CORES):
                    js = slice(NL * jc, NL * jc + NL)
                    pd = pdpool.tile([128, NL], F32, tag="pd")
                    py = pypool.tile([128, NL], F32, tag="py")
                    nc.tensor.matmul(pd[:], zm2r[:, rs], zfT[:, jc, :],
                                     start=True, stop=False)
                    nc.tensor.matmul(pd[:], sqones[:, rs], onesqf[:, js],
                                     start=False, stop=True)
                    nc.tensor.matmul(py[:], abl[:, rs], abf[:, js],
                                     start=True, stop=True)
                    c2 = lwpool.tile([128, NL], F32, tag="c2")
                    nc.vector.tensor_scalar_max(c2[:], pd[:], 0.0)
                    dd = lwpool.tile([128, NL], F32, tag="dd")
                    nc.scalar.activation(dd[:], c2[:], ACT.Sqrt)
                    tt = lwpool.tile([128, NL], F32, tag="tt")
                    nc.scalar.activation(tt[:], dd[:], ACT.Relu,
                                         bias=1.0, scale=-1.0)
                    cl = lwpool.tile([128, NL], F32, tag="cl")
                    nc.vector.select(
                        cl[:], py[:].bitcast(mybir.dt.int32),
                        dd[:], tt[:])
                    nc.sync.dma_start(out_d[rs, js], cl[:])


def _prep_inputs(samples, samples_info, conv1_w, conv1_b, conv2_w, conv2_b,
                 conv3_w, conv3_b, fc_w, fc_b, bn_gamma, bn_beta, bn_mean,
                 bn_var):
    f = np.float32
    samples = np.asarray(samples, f)
    info = np.asarray(samples_info, f)
    conv1_w = np.asarray(conv1_w, f)
    conv2_w = np.asarray(conv2_w, f)
    conv3_w = np.asarray(conv3_w, f)

    assert np.all(np.asarray(conv1_b) == 0), "conv1_b != 0 unsupported"
    assert np.all(np.asarray(conv2_b) == 0), "conv2_b != 0 unsupported"
    assert np.all(np.asarray(conv3_b) == 0), "conv3_b != 0 unsupported"

    # conv1 shifted weights, position pairs (l, l+2) packed into M=128:
    # cols 0-63 use shift s, cols 64-127 use shift s+2.  Indices 27/28 are
    # the left-only (shift 27/28) variants, 29/30 right-only (shift 0/1)
    # for pairs whose two windows land in adjacent x chunks.
    w1b = np.zeros((SIG, 128, C1), f)
    for s in range(SIG):
        w1b[s, s:s + K1, :] = conv1_w[:, 0, :].T
    w1s = np.zeros((31, 128, 128), f)
    for s in range(27):
        w1s[s, :, 0:64] = w1b[s]
        w1s[s, :, 64:128] = w1b[s + 2]
    for d in range(2):
        w1s[27 + d, :, 0:64] = w1b[27 + d]
        w1s[29 + d, :, 64:128] = w1b[d]
    # conv2 tap-pair weight variants (top rows 0-63 = even-parity tap,
    # bottom rows 64-127 = odd-parity tap of the same h1 pair slot):
    # even l2: V0=[t0;t1] V1=[t2;t3] V2=[t4;0]
    # odd  l2: V3=[0;t0]  V4=[t1;t2] V5=[t3;t4]
    w2t = [conv2_w[:, :, t].T for t in range(K2)]   # [64 ic, 128 oc]
    w2d = np.zeros((6, 128, C2), f)
    pairs = [(0, 1), (2, 3), (4, None), (None, 0), (1, 2), (3, 4)]
    for v, (top, bot) in enumerate(pairs):
        if top is not None:
            w2d[v, 0:64, :] = w2t[top]
        if bot is not None:
            w2d[v, 64:128, :] = w2t[bot]
    w3tt = np.zeros((K3, 128, C3), f)
    for t in range(K3):
        w3tt[t] = conv3_w[:, :, t].T   # [128 ic, 256 oc]
    fcw = np.zeros((2, 128, 128), f)
    fcwT = np.asarray(fc_w, f).T / f(T3)   # [256, 128]
    fcw[0] = fcwT[0:128, :]
    fcw[1] = fcwT[128:256, :]
    fcb = np.asarray(fc_b, f).reshape(128, 1)
    bna = (np.asarray(bn_gamma, f) /
           np.sqrt(np.asarray(bn_var, f) + f(1e-5))).reshape(128, 1)
    bnb = (np.asarray(bn_beta, f) -
           np.asarray(bn_mean, f).reshape(128) * bna[:, 0]).reshape(128, 1)

    writer, gen = info[:, 0], info[:, 1]
    assert np.all((writer == 0) | (writer == 1)), "non-binary writer id"
    a_full = (gen * (1.0 - writer)).astype(f)
    b_full = (gen * writer).astype(f)
    abf = np.stack([a_full, b_full])          # [2, N]

    import ml_dtypes
    bf = ml_dtypes.bfloat16
    w1s_b = w1s.astype(bf)
    w2d_b = w2d.astype(bf)
    w3t_b = w3tt.astype(bf)
    fcw_b = fcw.astype(bf)

    ones_col_np = np.ones((128, 1), f)
    ones_row_np = np.ones((1, N), f)

    # x transposed, padded (49 left / 50 right + tail), cut into 18
    # overlapping 128-row chunks at stride 29
    in_maps = []
    for core in range(N_CORES):
        n0 = core * NL
        xpad = np.zeros((624, NL), f)
        xpad[49:49 + L, :] = samples[n0:n0 + NL, 0, :].T
        xsc = np.zeros((NCHUNK1, 128, NL), f)
        for c in range(NCHUNK1):
            xsc[c] = xpad[SIG * c:SIG * c + 128, :]
        in_maps.append({
            "xs": xsc.astype(bf), "onc": ones_col_np, "onr": ones_row_np,
            "w1s": w1s_b, "w2d": w2d_b, "w3t": w3t_b, "fcw": fcw_b,
            "fcb": fcb,
            "bna": bna, "bnb": bnb,
            "abl": np.ascontiguousarray(abf[:, n0:n0 + NL]).astype(bf),
            "abf": abf.astype(bf),
        })
    return in_maps


def kernel(**inputs):
    global LAST_RESULT
    in_maps = _prep_inputs(**inputs)
    nc = build_nc()
    res = run_bass_kernel_spmd(nc, in_maps, core_ids=list(range(N_CORES)))
    LAST_RESULT = res
    out = np.concatenate([r["out"] for r in res.results], axis=0)
    np.fill_diagonal(out, 0.0)
    return out.astype(np.float32)
